# revision 42
# baseline (speedup 1.0000x reference)
import sys

sys.path.insert(0, "/opt/trn_rl_repo")
import numpy as np
import concourse.bass as bass
import concourse.tile as tile
from concourse import bacc, mybir
from concourse.bass_utils import run_bass_kernel_spmd

F32 = mybir.dt.float32
F32R = mybir.dt.float32r
AF = mybir.ActivationFunctionType

B, T, C = 64, 500, 256
E, H, D = 512, 8, 64
N_CORES = 8
BL = B // N_CORES  # batches per core

USE_F32R = True  # matmul dtype switch (f32r: 4x faster, ~2e-4 matmul rel err)

TT = [128, 128, 128, 116]  # t/s tile sizes (500 = 3*128 + 116)

EXP_SPLIT = False  # two 512-col exp ACTs per pst tile vs one 1024-col


MM_DT = F32R if USE_F32R else F32


def _mm(ap):
    # reinterpret as the matmul dtype (no-op when already MM_DT / fp32 mode)
    return ap if ap.dtype == MM_DT else ap.bitcast(MM_DT)


def build_nc():
    nc = bacc.Bacc("TRN2", target_bir_lowering=False)
    xt = nc.dram_tensor("xt", [BL, C, T], F32, kind="ExternalInput")
    wat = nc.dram_tensor("wat", [C, 3 * E], F32, kind="ExternalInput")  # w_attn.T
    wpt = nc.dram_tensor("wpt", [E, E], F32, kind="ExternalInput")  # w_proj.T
    bqk = nc.dram_tensor("bqk", [128, 8], F32, kind="ExternalInput")
    bvb = nc.dram_tensor("bvb", [128, E], F32, kind="ExternalInput")
    bpb = nc.dram_tensor("bpb", [128, E], F32, kind="ExternalInput")
    out = nc.dram_tensor("out", [BL, T, E], F32, kind="ExternalOutput")

    with tile.TileContext(nc) as tc:
        _build_body(nc, tc, xt, wat, wpt, bqk, bvb, bpb, out)
    nc.compile()
    return nc


def _build_body(nc, tc, xt, wat, wpt, bqk, bvb, bpb, out):
    from contextlib import ExitStack

    ctx = ExitStack()
    with ctx:
        cpool = ctx.enter_context(tc.tile_pool(name="consts", bufs=1))
        xpool = ctx.enter_context(tc.tile_pool(name="x", bufs=2))
        qkpool = ctx.enter_context(tc.tile_pool(name="qk", bufs=2))
        vpool = ctx.enter_context(tc.tile_pool(name="v", bufs=2))
        epool = ctx.enter_context(tc.tile_pool(name="est", bufs=3))
        ypool = ctx.enter_context(tc.tile_pool(name="yt", bufs=2))
        opool = ctx.enter_context(tc.tile_pool(name="os", bufs=2))
        zpool = ctx.enter_context(tc.tile_pool(name="zr", bufs=6))
        # PSUM pools: 8 banks total
        ps_io = ctx.enter_context(tc.tile_pool(name="ps_io", bufs=2, space="PSUM"))
        ps_st = ctx.enter_context(tc.tile_pool(name="ps_st", bufs=2, space="PSUM"))
        ps_yt = ctx.enter_context(tc.tile_pool(name="ps_yt", bufs=2, space="PSUM"))

        # ---- constants ----
        wa = cpool.tile([128, 2 * 3 * E], MM_DT, name="wa")  # 2 c-ktiles x [128,1536]
        for k in range(2):
            nc.sync.dma_start(wa[:, k * 1536:(k + 1) * 1536], _mm(wat[k * 128:(k + 1) * 128, :]))
        wp = cpool.tile([128, 4 * E], MM_DT, name="wp")  # 4 e-ktiles x [128,512]
        for k in range(4):
            nc.sync.dma_start(wp[:, k * E:(k + 1) * E], _mm(wpt[k * 128:(k + 1) * 128, :]))
        bqk_t = cpool.tile([128, 8], F32, name="bqk_t")
        nc.sync.dma_start(bqk_t[:], bqk[:, :])
        bvb_t = cpool.tile([128, E], F32, name="bvb_t")
        nc.sync.dma_start(bvb_t[:], bvb[:, :])
        bpb_t = cpool.tile([128, E], F32, name="bpb_t")
        nc.sync.dma_start(bpb_t[:], bpb[:, :])
        ones8 = cpool.tile([128, 8], F32, name="ones8")
        nc.vector.memset(ones8[:], 1.0)

        def do_proj(yt, b):
            # out[t,f] = yT^T @ wpT + bproj
            osb = opool.tile([128, 4 * E], F32, name=f"osb{b}", tag="osb")
            for mt in range(4):
                tt = TT[mt]
                po = ps_io.tile([128, E], F32, name=f"po{b}_{mt}", tag="ps_io")
                for k in range(4):
                    nc.tensor.matmul(
                        po[0:tt, :],
                        _mm(yt[:, k * T + mt * 128:k * T + mt * 128 + tt]),
                        _mm(wp[:, k * E:(k + 1) * E]),
                        start=(k == 0), stop=(k == 3),
                    )
                nc.vector.tensor_add(osb[0:tt, mt * E:(mt + 1) * E], po[0:tt, :], bpb_t[0:tt, :])
                nc.sync.dma_start(out[b, mt * 128:mt * 128 + tt, :], osb[0:tt, mt * E:(mt + 1) * E])

        def load_x(b):
            # t-axis padded to stride 512 (pad cols zeroed) so downstream
            # matmuls can use full 512-col moving operands
            xtb = xpool.tile([128, 2 * 512], MM_DT, name=f"xtb{b}", tag="xtb")
            for k in range(2):
                nc.sync.dma_start(xtb[:, k * 512:k * 512 + T], _mm(xt[b, k * 128:(k + 1) * 128, :]))
                pad = xtb[:, k * 512 + T:(k + 1) * 512]
                nc.vector.memset(pad if pad.dtype == F32 else pad.bitcast(F32), 0.0)
            return xtb

        xtb_next = load_x(0)
        prev = None  # (yt, b) awaiting projection
        for b in range(BL):
            xtb = xtb_next

            # ---- qkv(b): pq (ACT-drained) and pv (DVE-drained) interleaved so
            # the two ps_io slots recycle through different engines ----
            qk = qkpool.tile([128, 8 * 512], MM_DT, name=f"qk{b}", tag="qk")
            va = vpool.tile([128, 4 * 520], MM_DT, name=f"va{b}", tag="va")

            def do_pq(m):
                pq = ps_io.tile([128, 512], F32, name=f"pq{b}_{m}", tag="ps_io")
                for k in range(2):
                    nc.tensor.matmul(
                        pq[:],
                        _mm(wa[:, k * 1536 + m * 128:k * 1536 + (m + 1) * 128]),
                        _mm(xtb[:, k * 512:(k + 1) * 512]),
                        start=(k == 0), stop=(k == 1),
                    )
                # add per-partition bias (b_attn for q/k) while copying to SBUF
                nc.scalar.activation(qk[:, m * 512:(m + 1) * 512], pq[:], AF.Identity,
                                     bias=bqk_t[:, m:m + 1])

            def do_pv(mt):
                tt = TT[mt]
                pv = ps_io.tile([128, E], F32, name=f"pv{b}_{mt}", tag="ps_io")
                for k in range(2):
                    nc.tensor.matmul(
                        pv[0:tt, :],
                        _mm(xtb[:, k * 512 + mt * 128:k * 512 + mt * 128 + tt]),
                        _mm(wa[:, k * 1536 + 1024:k * 1536 + 1536]),
                        start=(k == 0), stop=(k == 1),
                    )
                va3 = va[:, mt * 520:(mt + 1) * 520].rearrange("p (h m) -> p h m", h=H)
                nc.vector.tensor_add(
                    va3[0:tt, :, 0:64],
                    pv[0:tt, :].rearrange("p (h m) -> p h m", h=H),
                    bvb_t[0:tt, :].rearrange("p (h m) -> p h m", h=H),
                )
                nc.vector.tensor_copy(
                    va3[:, :, 64:65], ones8[:].rearrange("p (h o) -> p h o", o=1)
                )

            # q/k e-tile order (p, 4+p) so ST pair p's operands complete early
            for m in (0, 4, 1, 5, 2, 6, 3, 7):
                do_pq(m)
            for mt in range(4):
                do_pv(mt)

            if b + 1 < BL:
                xtb_next = load_x(b + 1)

            if prev is not None:
                do_proj(*prev)

            # ---- attention heads, z-chain pipelined 2 deep ----
            est = {}
            pyt = {}
            zr = {}
            zbs = {}

            SW = 1024  # per-s-tile col stride in the pair-packed est (2 x 512)

            def do_st_half(h0, h1, srange):
                # ST[s,t] = k_h @ q_h^T; exp(ST/8) -> est (heads h0/h1 packed
                # side by side in the free dim so one ACT covers both)
                hp = h0 // 2
                if srange[0] == 0:
                    e2 = epool.tile([128, 4 * SW], MM_DT, name=f"est{b}_{hp}", tag="est")
                    est[h0] = e2
                    est[h1] = e2
                else:
                    e2 = est[h0]
                for s in srange:
                    st = TT[s]
                    pst = ps_st.tile([128, SW], F32, name=f"pst{b}_{hp}_{s}", tag="ps_st")
                    for idx, h in enumerate((h0, h1)):
                        jq, oq = h // 2, (h % 2) * 64
                        jk, ok = 4 + h // 2, (h % 2) * 64
                        nc.tensor.matmul(
                            pst[0:st, idx * 512:(idx + 1) * 512],
                            _mm(qk[ok:ok + 64, jk * 512 + s * 128:jk * 512 + s * 128 + st]),
                            _mm(qk[oq:oq + 64, jq * 512:(jq + 1) * 512]),
                            start=True, stop=True,
                        )
                    if EXP_SPLIT:
                        for idx in range(2):
                            nc.scalar.activation(
                                e2[0:st, s * SW + idx * 512:s * SW + (idx + 1) * 512],
                                pst[0:st, idx * 512:(idx + 1) * 512],
                                AF.Exp, scale=0.125)
                    else:
                        nc.scalar.activation(e2[0:st, s * SW:(s + 1) * SW],
                                             pst[0:st, :], AF.Exp, scale=0.125)

            def do_av_mm(h, s):
                # one s-chunk of yT[d,t] (+ z in row 64) = [v_h | 1]^T @ expST
                if s == 0:
                    pyt[h] = ps_yt.tile([65, T], F32, name=f"pyt{b}_{h}", tag="ps_yt")
                p = pyt[h]
                e = est[h]
                off = (h % 2) * 512
                st = TT[s]
                nc.tensor.matmul(
                    p[:],
                    _mm(va[0:st, s * 520 + 65 * h:s * 520 + 65 * h + 65]),
                    _mm(e[0:st, s * SW + off:s * SW + off + T]),
                    start=(s == 0), stop=(s == 3),
                )

            def do_z(h):
                p = pyt[h]
                z = zpool.tile([1, T], F32, name=f"zr{b}_{h}", tag="zr")
                zr[h] = z
                nc.vector.reciprocal(z[:], p[64:65, :])
                zs = zpool.tile([64, T], F32, name=f"zbs{b}_{h}", tag="zbs")
                zbs[h] = zs
                nc.gpsimd.partition_broadcast(zs[:], z[:])

            def do_yt(h):
                for s in range(4):
                    do_av_mm(h, s)
                do_z(h)

            def do_norm(h, yt):
                j, o = h // 2, (h % 2) * 64
                nc.vector.tensor_mul(
                    yt[o:o + 64, j * T:(j + 1) * T], pyt[h][0:64, :], zbs[h][:]
                )

            yt = ypool.tile([128, 4 * T], MM_DT, name=f"yt{b}", tag="yt")
            for hp in range(H // 2):
                # interleave previous pair's AV/norm between the ST halves so
                # PE has fill work while exp drains the pst tiles
                do_st_half(2 * hp, 2 * hp + 1, (0, 1))
                if hp >= 1:
                    do_yt(2 * hp - 2)
                    do_norm(2 * hp - 2, yt)
                do_st_half(2 * hp, 2 * hp + 1, (2, 3))
                if hp >= 1:
                    do_yt(2 * hp - 1)
                    do_norm(2 * hp - 1, yt)
            for h in (H - 2, H - 1):
                do_yt(h)
                do_norm(h, yt)

            prev = (yt, b)
        do_proj(*prev)


_NC = None


def _get_nc():
    global _NC
    if _NC is None:
        _NC = build_nc()
    return _NC


def prep_inputs(x, w_attn, b_attn, w_proj, b_proj):
    x = np.asarray(x, np.float32)
    w_attn = np.asarray(w_attn, np.float32)
    b_attn = np.asarray(b_attn, np.float32)
    w_proj = np.asarray(w_proj, np.float32)
    b_proj = np.asarray(b_proj, np.float32)

    xt_all = np.ascontiguousarray(x.transpose(0, 2, 1))  # [B, C, T]
    wat = np.ascontiguousarray(w_attn.T)  # [C, 1536]
    wpt = np.ascontiguousarray(w_proj.T)  # [E, E]
    bqk = np.ascontiguousarray(b_attn[:1024].reshape(8, 128).T)  # [128, 8]
    bvb = np.ascontiguousarray(np.tile(b_attn[1024:1536][None, :], (128, 1)))
    bpb = np.ascontiguousarray(np.tile(b_proj[None, :], (128, 1)))

    in_maps = []
    for c in range(N_CORES):
        in_maps.append({
            "xt": np.ascontiguousarray(xt_all[c * BL:(c + 1) * BL]),
            "wat": wat, "wpt": wpt, "bqk": bqk, "bvb": bvb, "bpb": bpb,
        })
    return in_maps


def kernel(x, w_attn, b_attn, w_proj, b_proj):
    nc = _get_nc()
    in_maps = prep_inputs(x, w_attn, b_attn, w_proj, b_proj)
    res = run_bass_kernel_spmd(nc, in_maps, core_ids=list(range(N_CORES)))
    out = np.concatenate([res.results[c]["out"] for c in range(N_CORES)], axis=0)
    return out.astype(np.float32)



# revision 43
# speedup vs baseline: 1.0102x; 1.0102x over previous
import sys

sys.path.insert(0, "/opt/trn_rl_repo")
import numpy as np
import concourse.bass as bass
import concourse.tile as tile
from concourse import bacc, mybir
from concourse.bass_utils import run_bass_kernel_spmd

F32 = mybir.dt.float32
F32R = mybir.dt.float32r
AF = mybir.ActivationFunctionType

B, T, C = 64, 500, 256
E, H, D = 512, 8, 64
N_CORES = 8
BL = B // N_CORES  # batches per core

USE_F32R = True  # matmul dtype switch (f32r: 4x faster, ~2e-4 matmul rel err)

TT = [128, 128, 128, 116]  # t/s tile sizes (500 = 3*128 + 116)

EXP_SPLIT = False  # two 512-col exp ACTs per pst tile vs one 1024-col


MM_DT = F32R if USE_F32R else F32


def _mm(ap):
    # reinterpret as the matmul dtype (no-op when already MM_DT / fp32 mode)
    return ap if ap.dtype == MM_DT else ap.bitcast(MM_DT)


def build_nc():
    nc = bacc.Bacc("TRN2", target_bir_lowering=False)
    xt = nc.dram_tensor("xt", [BL, C, T], F32, kind="ExternalInput")
    wat = nc.dram_tensor("wat", [C, 3 * E], F32, kind="ExternalInput")  # w_attn.T
    wpt = nc.dram_tensor("wpt", [E, E], F32, kind="ExternalInput")  # w_proj.T
    bqk = nc.dram_tensor("bqk", [128, 8], F32, kind="ExternalInput")
    bvb = nc.dram_tensor("bvb", [128, E], F32, kind="ExternalInput")
    bpb = nc.dram_tensor("bpb", [128, E], F32, kind="ExternalInput")
    out = nc.dram_tensor("out", [BL, T, E], F32, kind="ExternalOutput")

    with tile.TileContext(nc) as tc:
        _build_body(nc, tc, xt, wat, wpt, bqk, bvb, bpb, out)
    nc.compile()
    return nc


def _build_body(nc, tc, xt, wat, wpt, bqk, bvb, bpb, out):
    from contextlib import ExitStack

    ctx = ExitStack()
    with ctx:
        cpool = ctx.enter_context(tc.tile_pool(name="consts", bufs=1))
        xpool = ctx.enter_context(tc.tile_pool(name="x", bufs=2))
        qkpool = ctx.enter_context(tc.tile_pool(name="qk", bufs=2))
        vpool = ctx.enter_context(tc.tile_pool(name="v", bufs=2))
        epool = ctx.enter_context(tc.tile_pool(name="est", bufs=2))
        ypool = ctx.enter_context(tc.tile_pool(name="yt", bufs=2))
        opool = ctx.enter_context(tc.tile_pool(name="os", bufs=2))
        zpool = ctx.enter_context(tc.tile_pool(name="zr", bufs=3))
        # PSUM pools: 8 banks total
        ps_io = ctx.enter_context(tc.tile_pool(name="ps_io", bufs=2, space="PSUM"))
        ps_st = ctx.enter_context(tc.tile_pool(name="ps_st", bufs=2, space="PSUM"))
        ps_yt = ctx.enter_context(tc.tile_pool(name="ps_yt", bufs=2, space="PSUM"))

        # ---- constants ----
        wa = cpool.tile([128, 2 * 3 * E], MM_DT, name="wa")  # 2 c-ktiles x [128,1536]
        for k in range(2):
            nc.sync.dma_start(wa[:, k * 1536:(k + 1) * 1536], _mm(wat[k * 128:(k + 1) * 128, :]))
        wp = cpool.tile([128, 4 * E], MM_DT, name="wp")  # 4 e-ktiles x [128,512]
        for k in range(4):
            nc.sync.dma_start(wp[:, k * E:(k + 1) * E], _mm(wpt[k * 128:(k + 1) * 128, :]))
        bqk_t = cpool.tile([128, 8], F32, name="bqk_t")
        nc.sync.dma_start(bqk_t[:], bqk[:, :])
        bvb_t = cpool.tile([128, E], F32, name="bvb_t")
        nc.sync.dma_start(bvb_t[:], bvb[:, :])
        bpb_t = cpool.tile([128, E], F32, name="bpb_t")
        nc.sync.dma_start(bpb_t[:], bpb[:, :])
        ones8 = cpool.tile([128, 8], F32, name="ones8")
        nc.vector.memset(ones8[:], 1.0)

        def do_proj(yt, b):
            # out[t,f] = yT^T @ wpT + bproj
            osb = opool.tile([128, 4 * E], F32, name=f"osb{b}", tag="osb")
            for mt in range(4):
                tt = TT[mt]
                po = ps_io.tile([128, E], F32, name=f"po{b}_{mt}", tag="ps_io")
                for k in range(4):
                    nc.tensor.matmul(
                        po[0:tt, :],
                        _mm(yt[:, k * T + mt * 128:k * T + mt * 128 + tt]),
                        _mm(wp[:, k * E:(k + 1) * E]),
                        start=(k == 0), stop=(k == 3),
                    )
                nc.vector.tensor_add(osb[0:tt, mt * E:(mt + 1) * E], po[0:tt, :], bpb_t[0:tt, :])
                nc.sync.dma_start(out[b, mt * 128:mt * 128 + tt, :], osb[0:tt, mt * E:(mt + 1) * E])

        def load_x(b):
            # t-axis padded to stride 512 (pad cols zeroed) so downstream
            # matmuls can use full 512-col moving operands
            xtb = xpool.tile([128, 2 * 512], MM_DT, name=f"xtb{b}", tag="xtb")
            for k in range(2):
                nc.sync.dma_start(xtb[:, k * 512:k * 512 + T], _mm(xt[b, k * 128:(k + 1) * 128, :]))
                pad = xtb[:, k * 512 + T:(k + 1) * 512]
                nc.vector.memset(pad if pad.dtype == F32 else pad.bitcast(F32), 0.0)
            return xtb

        xtb_next = load_x(0)
        prev = None  # (yt, b) awaiting projection
        for b in range(BL):
            xtb = xtb_next

            # ---- qkv(b): pq (ACT-drained) and pv (DVE-drained) interleaved so
            # the two ps_io slots recycle through different engines ----
            qk = qkpool.tile([128, 8 * 512], MM_DT, name=f"qk{b}", tag="qk")
            va = vpool.tile([128, 4 * 520], MM_DT, name=f"va{b}", tag="va")

            def do_pq(m):
                pq = ps_io.tile([128, 512], F32, name=f"pq{b}_{m}", tag="ps_io")
                for k in range(2):
                    nc.tensor.matmul(
                        pq[:],
                        _mm(wa[:, k * 1536 + m * 128:k * 1536 + (m + 1) * 128]),
                        _mm(xtb[:, k * 512:(k + 1) * 512]),
                        start=(k == 0), stop=(k == 1),
                    )
                # add per-partition bias (b_attn for q/k) while copying to SBUF
                nc.scalar.activation(qk[:, m * 512:(m + 1) * 512], pq[:], AF.Identity,
                                     bias=bqk_t[:, m:m + 1])

            def do_pv(mt):
                tt = TT[mt]
                pv = ps_io.tile([128, E], F32, name=f"pv{b}_{mt}", tag="ps_io")
                for k in range(2):
                    nc.tensor.matmul(
                        pv[0:tt, :],
                        _mm(xtb[:, k * 512 + mt * 128:k * 512 + mt * 128 + tt]),
                        _mm(wa[:, k * 1536 + 1024:k * 1536 + 1536]),
                        start=(k == 0), stop=(k == 1),
                    )
                va3 = va[:, mt * 520:(mt + 1) * 520].rearrange("p (h m) -> p h m", h=H)
                nc.vector.tensor_add(
                    va3[0:tt, :, 0:64],
                    pv[0:tt, :].rearrange("p (h m) -> p h m", h=H),
                    bvb_t[0:tt, :].rearrange("p (h m) -> p h m", h=H),
                )
                nc.vector.tensor_copy(
                    va3[:, :, 64:65], ones8[:].rearrange("p (h o) -> p h o", o=1)
                )

            # q/k e-tile order (p, 4+p) so ST pair p's operands complete early
            for m in (0, 4, 1, 5, 2, 6, 3, 7):
                do_pq(m)
            for mt in range(4):
                do_pv(mt)

            if b + 1 < BL:
                xtb_next = load_x(b + 1)

            # ---- attention heads, z-chain pipelined 2 deep ----
            est = {}
            pyt = {}
            zr = {}
            zbs = {}

            SW = 1024  # per-s-tile col stride in the pair-packed est (2 x 512)

            def do_st_half(h0, h1, srange):
                # ST[s,t] = k_h @ q_h^T; exp(ST/8) -> est (heads h0/h1 packed
                # side by side in the free dim so one ACT covers both)
                hp = h0 // 2
                if srange[0] == 0:
                    e2 = epool.tile([128, 4 * SW], MM_DT, name=f"est{b}_{hp}", tag="est")
                    est[h0] = e2
                    est[h1] = e2
                else:
                    e2 = est[h0]
                for s in srange:
                    st = TT[s]
                    pst = ps_st.tile([128, SW], F32, name=f"pst{b}_{hp}_{s}", tag="ps_st")
                    for idx, h in enumerate((h0, h1)):
                        jq, oq = h // 2, (h % 2) * 64
                        jk, ok = 4 + h // 2, (h % 2) * 64
                        nc.tensor.matmul(
                            pst[0:st, idx * 512:(idx + 1) * 512],
                            _mm(qk[ok:ok + 64, jk * 512 + s * 128:jk * 512 + s * 128 + st]),
                            _mm(qk[oq:oq + 64, jq * 512:(jq + 1) * 512]),
                            start=True, stop=True,
                        )
                    if EXP_SPLIT:
                        for idx in range(2):
                            nc.scalar.activation(
                                e2[0:st, s * SW + idx * 512:s * SW + (idx + 1) * 512],
                                pst[0:st, idx * 512:(idx + 1) * 512],
                                AF.Exp, scale=0.125)
                    else:
                        nc.scalar.activation(e2[0:st, s * SW:(s + 1) * SW],
                                             pst[0:st, :], AF.Exp, scale=0.125)

            def do_av_mm(h, s):
                # one s-chunk of yT[d,t] (+ z in row 64) = [v_h | 1]^T @ expST
                if s == 0:
                    pyt[h] = ps_yt.tile([65, T], F32, name=f"pyt{b}_{h}", tag="ps_yt")
                p = pyt[h]
                e = est[h]
                off = (h % 2) * 512
                st = TT[s]
                nc.tensor.matmul(
                    p[:],
                    _mm(va[0:st, s * 520 + 65 * h:s * 520 + 65 * h + 65]),
                    _mm(e[0:st, s * SW + off:s * SW + off + T]),
                    start=(s == 0), stop=(s == 3),
                )

            def do_z(h):
                p = pyt[h]
                z = zpool.tile([1, T], F32, name=f"zr{b}_{h}", tag="zr")
                zr[h] = z
                nc.vector.reciprocal(z[:], p[64:65, :])
                zs = zpool.tile([64, T], F32, name=f"zbs{b}_{h}", tag="zbs")
                zbs[h] = zs
                nc.gpsimd.partition_broadcast(zs[:], z[:])

            def do_yt(h):
                for s in range(4):
                    do_av_mm(h, s)
                do_z(h)

            def do_norm(h, yt):
                j, o = h // 2, (h % 2) * 64
                nc.vector.tensor_mul(
                    yt[o:o + 64, j * T:(j + 1) * T], pyt[h][0:64, :], zbs[h][:]
                )

            yt = ypool.tile([128, 4 * T], MM_DT, name=f"yt{b}", tag="yt")
            for hp in range(H // 2):
                # interleave previous pair's AV/norm between the ST halves so
                # PE has fill work while exp drains the pst tiles; prev batch's
                # projection is emitted after the first ST so attention starts
                # as early as possible
                do_st_half(2 * hp, 2 * hp + 1, (0, 1))
                if hp == 0 and prev is not None:
                    do_proj(*prev)
                if hp >= 1:
                    do_yt(2 * hp - 2)
                    do_norm(2 * hp - 2, yt)
                do_st_half(2 * hp, 2 * hp + 1, (2, 3))
                if hp >= 1:
                    do_yt(2 * hp - 1)
                    do_norm(2 * hp - 1, yt)
            for h in (H - 2, H - 1):
                do_yt(h)
                do_norm(h, yt)

            prev = (yt, b)
        do_proj(*prev)


_NC = None


def _get_nc():
    global _NC
    if _NC is None:
        _NC = build_nc()
    return _NC


def prep_inputs(x, w_attn, b_attn, w_proj, b_proj):
    x = np.asarray(x, np.float32)
    w_attn = np.asarray(w_attn, np.float32)
    b_attn = np.asarray(b_attn, np.float32)
    w_proj = np.asarray(w_proj, np.float32)
    b_proj = np.asarray(b_proj, np.float32)

    xt_all = np.ascontiguousarray(x.transpose(0, 2, 1))  # [B, C, T]
    wat = np.ascontiguousarray(w_attn.T)  # [C, 1536]
    wpt = np.ascontiguousarray(w_proj.T)  # [E, E]
    bqk = np.ascontiguousarray(b_attn[:1024].reshape(8, 128).T)  # [128, 8]
    bvb = np.ascontiguousarray(np.tile(b_attn[1024:1536][None, :], (128, 1)))
    bpb = np.ascontiguousarray(np.tile(b_proj[None, :], (128, 1)))

    in_maps = []
    for c in range(N_CORES):
        in_maps.append({
            "xt": np.ascontiguousarray(xt_all[c * BL:(c + 1) * BL]),
            "wat": wat, "wpt": wpt, "bqk": bqk, "bvb": bvb, "bpb": bpb,
        })
    return in_maps


def kernel(x, w_attn, b_attn, w_proj, b_proj):
    nc = _get_nc()
    in_maps = prep_inputs(x, w_attn, b_attn, w_proj, b_proj)
    res = run_bass_kernel_spmd(nc, in_maps, core_ids=list(range(N_CORES)))
    out = np.concatenate([res.results[c]["out"] for c in range(N_CORES)], axis=0)
    return out.astype(np.float32)



# revision 44
# speedup vs baseline: 1.1230x; 1.1116x over previous
import sys

sys.path.insert(0, "/opt/trn_rl_repo")
import numpy as np
import concourse.bass as bass
import concourse.tile as tile
from concourse import bacc, mybir
from concourse.bass_utils import run_bass_kernel_spmd

F32 = mybir.dt.float32
F32R = mybir.dt.float32r
AF = mybir.ActivationFunctionType

B, T, C = 64, 500, 256
E, H, D = 512, 8, 64
N_CORES = 8
BL = B // N_CORES  # batches per core

USE_F32R = True  # matmul dtype switch (f32r: 4x faster, ~2e-4 matmul rel err)

TT = [128, 128, 128, 116]  # t/s tile sizes (500 = 3*128 + 116)

EXP_SPLIT = False  # two 512-col exp ACTs per pst tile vs one 1024-col


MM_DT = F32R if USE_F32R else F32


def _mm(ap):
    # reinterpret as the matmul dtype (no-op when already MM_DT / fp32 mode)
    return ap if ap.dtype == MM_DT else ap.bitcast(MM_DT)


def build_nc():
    nc = bacc.Bacc("TRN2", target_bir_lowering=False)
    xt = nc.dram_tensor("xt", [BL, C, T], F32, kind="ExternalInput")
    wat = nc.dram_tensor("wat", [C, 3 * E], F32, kind="ExternalInput")  # w_attn.T
    wpt = nc.dram_tensor("wpt", [E, E], F32, kind="ExternalInput")  # w_proj.T
    bqk = nc.dram_tensor("bqk", [128, 8], F32, kind="ExternalInput")
    bvb = nc.dram_tensor("bvb", [128, E], F32, kind="ExternalInput")
    bpb = nc.dram_tensor("bpb", [128, E], F32, kind="ExternalInput")
    out = nc.dram_tensor("out", [BL, T, E], F32, kind="ExternalOutput")

    with tile.TileContext(nc) as tc:
        _build_body(nc, tc, xt, wat, wpt, bqk, bvb, bpb, out)
    nc.compile()
    return nc


def _build_body(nc, tc, xt, wat, wpt, bqk, bvb, bpb, out):
    from contextlib import ExitStack

    ctx = ExitStack()
    with ctx:
        cpool = ctx.enter_context(tc.tile_pool(name="consts", bufs=1))
        xpool = ctx.enter_context(tc.tile_pool(name="x", bufs=2))
        qkpool = ctx.enter_context(tc.tile_pool(name="qk", bufs=2))
        vpool = ctx.enter_context(tc.tile_pool(name="v", bufs=2))
        epool = ctx.enter_context(tc.tile_pool(name="est", bufs=2))
        ypool = ctx.enter_context(tc.tile_pool(name="yt", bufs=2))
        opool = ctx.enter_context(tc.tile_pool(name="os", bufs=2))
        zpool = ctx.enter_context(tc.tile_pool(name="zr", bufs=3))
        # PSUM pools: 8 banks total
        ps_io = ctx.enter_context(tc.tile_pool(name="ps_io", bufs=2, space="PSUM"))
        ps_st = ctx.enter_context(tc.tile_pool(name="ps_st", bufs=2, space="PSUM"))
        ps_yt = ctx.enter_context(tc.tile_pool(name="ps_yt", bufs=2, space="PSUM"))

        # ---- constants ----
        wa = cpool.tile([128, 2 * 3 * E], MM_DT, name="wa")  # 2 c-ktiles x [128,1536]
        for k in range(2):
            nc.sync.dma_start(wa[:, k * 1536:(k + 1) * 1536], _mm(wat[k * 128:(k + 1) * 128, :]))
        wp = cpool.tile([128, 4 * E], MM_DT, name="wp")  # 4 e-ktiles x [128,512]
        for k in range(4):
            nc.sync.dma_start(wp[:, k * E:(k + 1) * E], _mm(wpt[k * 128:(k + 1) * 128, :]))
        bqk_t = cpool.tile([128, 8], F32, name="bqk_t")
        nc.sync.dma_start(bqk_t[:], bqk[:, :])
        bvb_t = cpool.tile([128, E], F32, name="bvb_t")
        nc.sync.dma_start(bvb_t[:], bvb[:, :])
        bpb_t = cpool.tile([128, E], F32, name="bpb_t")
        nc.sync.dma_start(bpb_t[:], bpb[:, :])
        ones8 = cpool.tile([128, 8], F32, name="ones8")
        nc.vector.memset(ones8[:], 1.0)

        def do_proj(yt, b):
            # out[t,f] = yT^T @ wpT + bproj
            osb = opool.tile([128, 4 * E], F32, name=f"osb{b}", tag="osb")
            for mt in range(4):
                tt = TT[mt]
                po = ps_io.tile([128, E], F32, name=f"po{b}_{mt}", tag="ps_io")
                for k in range(4):
                    nc.tensor.matmul(
                        po[0:tt, :],
                        _mm(yt[:, k * T + mt * 128:k * T + mt * 128 + tt]),
                        _mm(wp[:, k * E:(k + 1) * E]),
                        start=(k == 0), stop=(k == 3),
                    )
                nc.vector.tensor_add(osb[0:tt, mt * E:(mt + 1) * E], po[0:tt, :], bpb_t[0:tt, :])
                nc.sync.dma_start(out[b, mt * 128:mt * 128 + tt, :], osb[0:tt, mt * E:(mt + 1) * E])

        def load_x(b):
            # t-axis padded to stride 512 (pad cols zeroed) so downstream
            # matmuls can use full 512-col moving operands
            xtb = xpool.tile([128, 2 * 512], MM_DT, name=f"xtb{b}", tag="xtb")
            for k in range(2):
                nc.sync.dma_start(xtb[:, k * 512:k * 512 + T], _mm(xt[b, k * 128:(k + 1) * 128, :]))
                pad = xtb[:, k * 512 + T:(k + 1) * 512]
                nc.vector.memset(pad if pad.dtype == F32 else pad.bitcast(F32), 0.0)
            return xtb

        xtb_next = load_x(0)
        prev = None  # (yt, b) awaiting projection
        for b in range(BL):
            xtb = xtb_next

            # ---- qkv(b): pq (ACT-drained) and pv (DVE-drained) interleaved so
            # the two ps_io slots recycle through different engines ----
            qk = qkpool.tile([128, 8 * 512], MM_DT, name=f"qk{b}", tag="qk")
            va = vpool.tile([128, 4 * 520], MM_DT, name=f"va{b}", tag="va")

            def do_pq(m):
                pq = ps_io.tile([128, 512], F32, name=f"pq{b}_{m}", tag="ps_io")
                for k in range(2):
                    nc.tensor.matmul(
                        pq[:],
                        _mm(wa[:, k * 1536 + m * 128:k * 1536 + (m + 1) * 128]),
                        _mm(xtb[:, k * 512:(k + 1) * 512]),
                        start=(k == 0), stop=(k == 1),
                    )
                # add per-partition bias (b_attn for q/k) while copying to SBUF
                nc.scalar.activation(qk[:, m * 512:(m + 1) * 512], pq[:], AF.Identity,
                                     bias=bqk_t[:, m:m + 1])

            def do_pv(mt):
                tt = TT[mt]
                pv = ps_io.tile([128, E], F32, name=f"pv{b}_{mt}", tag="ps_io")
                for k in range(2):
                    nc.tensor.matmul(
                        pv[0:tt, :],
                        _mm(xtb[:, k * 512 + mt * 128:k * 512 + mt * 128 + tt]),
                        _mm(wa[:, k * 1536 + 1024:k * 1536 + 1536]),
                        start=(k == 0), stop=(k == 1),
                    )
                va3 = va[:, mt * 520:(mt + 1) * 520].rearrange("p (h m) -> p h m", h=H)
                nc.vector.tensor_add(
                    va3[0:tt, :, 0:64],
                    pv[0:tt, :].rearrange("p (h m) -> p h m", h=H),
                    bvb_t[0:tt, :].rearrange("p (h m) -> p h m", h=H),
                )
                nc.vector.tensor_copy(
                    va3[:, :, 64:65], ones8[:].rearrange("p (h o) -> p h o", o=1)
                )

            # q/k e-tile order (p, 4+p) so ST pair p's operands complete early
            for m in (0, 4, 1, 5, 2, 6, 3, 7):
                do_pq(m)
            for mt in range(4):
                do_pv(mt)

            if b + 1 < BL:
                xtb_next = load_x(b + 1)

            if prev is not None:
                do_proj(*prev)

            # ---- attention heads, z-chain pipelined 2 deep ----
            est = {}
            pyt = {}
            zr = {}
            zbs = {}

            SW = 1024  # per-s-tile col stride in the pair-packed est (2 x 512)

            def do_st_half(h0, h1, srange):
                # ST[s,t] = k_h @ q_h^T; exp(ST/8) -> est (heads h0/h1 packed
                # side by side in the free dim so one ACT covers both)
                hp = h0 // 2
                if srange[0] == 0:
                    e2 = epool.tile([128, 4 * SW], MM_DT, name=f"est{b}_{hp}", tag="est")
                    est[h0] = e2
                    est[h1] = e2
                else:
                    e2 = est[h0]
                for s in srange:
                    st = TT[s]
                    pst = ps_st.tile([128, SW], F32, name=f"pst{b}_{hp}_{s}", tag="ps_st")
                    for idx, h in enumerate((h0, h1)):
                        jq, oq = h // 2, (h % 2) * 64
                        jk, ok = 4 + h // 2, (h % 2) * 64
                        nc.tensor.matmul(
                            pst[0:st, idx * 512:(idx + 1) * 512],
                            _mm(qk[ok:ok + 64, jk * 512 + s * 128:jk * 512 + s * 128 + st]),
                            _mm(qk[oq:oq + 64, jq * 512:(jq + 1) * 512]),
                            start=True, stop=True,
                        )
                    if EXP_SPLIT:
                        for idx in range(2):
                            nc.scalar.activation(
                                e2[0:st, s * SW + idx * 512:s * SW + (idx + 1) * 512],
                                pst[0:st, idx * 512:(idx + 1) * 512],
                                AF.Exp, scale=0.125)
                    else:
                        nc.scalar.activation(e2[0:st, s * SW:(s + 1) * SW],
                                             pst[0:st, :], AF.Exp, scale=0.125)

            def do_av_mm(h, s):
                # one s-chunk of yT[d,t] (+ z in row 64) = [v_h | 1]^T @ expST
                if s == 0:
                    pyt[h] = ps_yt.tile([65, T], F32, name=f"pyt{b}_{h}", tag="ps_yt")
                p = pyt[h]
                e = est[h]
                off = (h % 2) * 512
                st = TT[s]
                nc.tensor.matmul(
                    p[:],
                    _mm(va[0:st, s * 520 + 65 * h:s * 520 + 65 * h + 65]),
                    _mm(e[0:st, s * SW + off:s * SW + off + T]),
                    start=(s == 0), stop=(s == 3),
                )

            def do_z(h):
                p = pyt[h]
                z = zpool.tile([1, T], F32, name=f"zr{b}_{h}", tag="zr")
                zr[h] = z
                nc.vector.reciprocal(z[:], p[64:65, :])
                zs = zpool.tile([64, T], F32, name=f"zbs{b}_{h}", tag="zbs")
                zbs[h] = zs
                nc.gpsimd.partition_broadcast(zs[:], z[:])

            def do_yt(h):
                for s in range(4):
                    do_av_mm(h, s)
                do_z(h)

            def do_norm(h, yt):
                j, o = h // 2, (h % 2) * 64
                nc.vector.tensor_mul(
                    yt[o:o + 64, j * T:(j + 1) * T], pyt[h][0:64, :], zbs[h][:]
                )

            yt = ypool.tile([128, 4 * T], MM_DT, name=f"yt{b}", tag="yt")
            for hp in range(H // 2):
                # interleave previous pair's AV/norm between the ST halves so
                # PE has fill work while exp drains the pst tiles
                do_st_half(2 * hp, 2 * hp + 1, (0, 1))
                if hp >= 1:
                    do_yt(2 * hp - 2)
                    do_norm(2 * hp - 2, yt)
                do_st_half(2 * hp, 2 * hp + 1, (2, 3))
                if hp >= 1:
                    do_yt(2 * hp - 1)
                    do_norm(2 * hp - 1, yt)
            for h in (H - 2, H - 1):
                do_yt(h)
                do_norm(h, yt)

            prev = (yt, b)
        do_proj(*prev)


_NC = None


def _get_nc():
    global _NC
    if _NC is None:
        _NC = build_nc()
    return _NC


def prep_inputs(x, w_attn, b_attn, w_proj, b_proj):
    x = np.asarray(x, np.float32)
    w_attn = np.asarray(w_attn, np.float32)
    b_attn = np.asarray(b_attn, np.float32)
    w_proj = np.asarray(w_proj, np.float32)
    b_proj = np.asarray(b_proj, np.float32)

    xt_all = np.ascontiguousarray(x.transpose(0, 2, 1))  # [B, C, T]
    wat = np.ascontiguousarray(w_attn.T)  # [C, 1536]
    wpt = np.ascontiguousarray(w_proj.T)  # [E, E]
    bqk = np.ascontiguousarray(b_attn[:1024].reshape(8, 128).T)  # [128, 8]
    bvb = np.ascontiguousarray(np.tile(b_attn[1024:1536][None, :], (128, 1)))
    bpb = np.ascontiguousarray(np.tile(b_proj[None, :], (128, 1)))

    in_maps = []
    for c in range(N_CORES):
        in_maps.append({
            "xt": np.ascontiguousarray(xt_all[c * BL:(c + 1) * BL]),
            "wat": wat, "wpt": wpt, "bqk": bqk, "bvb": bvb, "bpb": bpb,
        })
    return in_maps


def kernel(x, w_attn, b_attn, w_proj, b_proj):
    nc = _get_nc()
    in_maps = prep_inputs(x, w_attn, b_attn, w_proj, b_proj)
    res = run_bass_kernel_spmd(nc, in_maps, core_ids=list(range(N_CORES)))
    out = np.concatenate([res.results[c]["out"] for c in range(N_CORES)], axis=0)
    return out.astype(np.float32)

